# revision 2
# baseline (speedup 1.0000x reference)
"""Trainium2 Bass kernel for nn_Anchor3 (segment_reduce): 8-core SPMD, v2.

Per core (125k nodes/branch, bf16 data path):
  - segment-sum on the TensorEngine: host sorts each shard by class and
    deals rows into 32 windows of 128 classes (33 chunks of 128 rows per
    window, zero-padded); per chunk the DVE builds a one-hot [row, cls]
    via is_equal(iota, cls_rel) and the PE contracts rows:
    psumT[64, 128] += sdat_chunk[128, 64].T @ onehot[128, 128]
    accumulating a transposed class-window sum directly (no transpose pass)
  - AllReduce the [2*64, 4096] f32 partial sums; multiply by host 1/(cnt+eps)
    to get feaT [64, 4096] per branch
  - class-level cross-attention (queries sharded 512/core): per-head padded
    weights, 4 heads as tile-position matmuls, fused sum-of-exp column,
    division deferred past the V-contraction (same scheme as v1)
  - attention output rows are cast to bf16 and QUAD-replICATED into a
    [1024, 256] table slice; one AllGather per branch -> [8192, 256] table
  - output gather: host groups same-class rows into quads; SWDGE dma_gather
    (elem=512B, 1024 idx/call, 4 queues) pulls one 4-row quad per index;
    bulk stores write [128, 32, 256] bf16 pieces; host un-sorts.
Host does index-metadata prep only (sort, counts, quad layout) plus the
bf16 casts that define the kernel's working precision.
"""
import functools
import os

import numpy as np
import ml_dtypes

import concourse.bass as bass
import concourse.bacc as bacc
import concourse.mybir as mybir
import concourse.tile as tile
from concourse import library_config
from concourse.bass_utils import run_bass_kernel_spmd

N_CORES = 8
NV = 1_000_000
VN = 4096          # classes per branch
E = 64
H = 4
HD = 16
SHARD = NV // N_CORES            # 125000

WIN = 32                         # class windows per branch (128 classes each)
WCAP = 33                        # chunk capacity per window (33*128=4224 rows)
NCHK = WIN * WCAP                # 1056 chunks per branch
NSLOT = NCHK * 128               # 135168 row slots
PIECE_CH = 4 * WCAP              # chunks per load piece = 4 windows = 132
NPIECE = NCHK // PIECE_CH        # 8

GC = 1024                        # gather idx per call (SWDGE ring-safe)
QELEM = 256                      # bf16 elems per gathered quad (512B)
CALLS_PER_PIECE = 4              # 4 gather calls -> one 2MB store
QCH = VN // N_CORES              # 512 query rows per core
TC = VN // 128                   # 32 key chunks in attention

DT = mybir.dt.float32
BF = mybir.dt.bfloat16
I16 = mybir.dt.int16
BF_NP = ml_dtypes.bfloat16


def _build(nq_slots: int):
    """nq_slots: gather quads per branch (multiple of CALLS_PER_PIECE*GC)."""
    skip_pha = bool(os.environ.get("KSKIP_PHA"))
    skip_at = bool(os.environ.get("KSKIP_ATTN"))
    skip_ga = bool(os.environ.get("KSKIP_GA"))
    skip_ar = bool(os.environ.get("KSKIP_AR"))
    skip_ld = bool(os.environ.get("KSKIP_LOADS"))
    nq_piece = CALLS_PER_PIECE * GC              # quads per store piece
    n_gpiece = nq_slots // nq_piece

    nc = bacc.Bacc("TRN2", num_swdge_queues=4)

    ins = {}
    for br in ("v", "c"):
        ins[f"sdat_{br}"] = nc.declare_dram_parameter(
            f"sdat_{br}", [128, NCHK, E], BF, isOutput=False)
        ins[f"crel_{br}"] = nc.declare_dram_parameter(
            f"crel_{br}", [128, NCHK], DT, isOutput=False)
        ins[f"invT_{br}"] = nc.declare_dram_parameter(
            f"invT_{br}", [E, VN], DT, isOutput=False)
        ins[f"gidx_{br}"] = nc.declare_dram_parameter(
            f"gidx_{br}", [128, nq_slots // 16], I16, isOutput=False)
        ins[f"semq_{br}"] = nc.declare_dram_parameter(
            f"semq_{br}", [E, QCH], DT, isOutput=False)
        ins[f"wqT_{br}"] = nc.declare_dram_parameter(f"wqT_{br}", [E, 128], DT, isOutput=False)
        ins[f"wkT_{br}"] = nc.declare_dram_parameter(f"wkT_{br}", [E, 128], DT, isOutput=False)
        ins[f"wvT_{br}"] = nc.declare_dram_parameter(f"wvT_{br}", [E, E], DT, isOutput=False)
        ins[f"woT_{br}"] = nc.declare_dram_parameter(f"woT_{br}", [E, E], DT, isOutput=False)
        ins[f"bq_{br}"] = nc.declare_dram_parameter(f"bq_{br}", [128, 1], DT, isOutput=False)
        ins[f"bk_{br}"] = nc.declare_dram_parameter(f"bk_{br}", [128, 1], DT, isOutput=False)
        ins[f"bv_{br}"] = nc.declare_dram_parameter(f"bv_{br}", [E, 1], DT, isOutput=False)
        ins[f"bo_{br}"] = nc.declare_dram_parameter(f"bo_{br}", [E, 1], DT, isOutput=False)
    ident = nc.declare_dram_parameter("ident", [128, 128], DT, isOutput=False)
    iotaF = nc.declare_dram_parameter("iotaF", [128, 128], BF, isOutput=False)
    out_ext = nc.declare_dram_parameter(
        "out", [2, n_gpiece, 128, CALLS_PER_PIECE * (GC // 128), QELEM], BF,
        isOutput=True)

    acc = nc.dram_tensor("acc", [2 * E, VN], DT)
    acc_red = nc.dram_tensor("acc_red", [2 * E, VN], DT, addr_space="Shared")
    tbl_own = nc.dram_tensor("tbl_own", [2 * QCH, QELEM], BF)
    tbl_all = nc.dram_tensor("tbl_all", [N_CORES * 2 * QCH, QELEM], BF,
                             addr_space="Shared")

    rg = [list(range(N_CORES))]

    with tile.TileContext(nc) as tc:
        nc.gpsimd.load_library(library_config.mlp)

        with tc.tile_pool(name="cst", bufs=1) as cst:
            iot = cst.tile([128, 128], BF, name="iot")
            nc.sync.dma_start(out=iot[:], in_=iotaF[:])
            idt = cst.tile([128, 128], DT, name="idt")
            nc.sync.dma_start(out=idt[:], in_=ident[:])
            crel = {}
            for br in ("v", "c"):
                crel[br] = cst.tile([128, NCHK], DT, name=f"crel_{br}")
                nc.sync.dma_start(out=crel[br][:], in_=ins[f"crel_{br}"][:])

            # ---------------- phase A: per-branch PE segment sums ----------
            with tc.tile_pool(name="sums", bufs=1) as smp:
                sumsT = {br: smp.tile([E, VN], DT, name=f"sumsT_{br}")
                         for br in ("v", "c")}
                for br in ("v", "c"):
                    if skip_pha:
                        nc.vector.memset(sumsT[br][:], 0.0)
                        nc.sync.dma_start(
                            out=acc[(0 if br == "v" else E):
                                    (E if br == "v" else 2 * E), :],
                            in_=sumsT[br][:])
                        continue
                    sdat = ins[f"sdat_{br}"]
                    with tc.tile_pool(name=f"ld_{br}", bufs=3) as ldp, \
                         tc.tile_pool(name=f"oh_{br}", bufs=4) as ohp, \
                         tc.tile_pool(name=f"psA_{br}", bufs=2,
                                      space=bass.MemorySpace.PSUM) as psA:
                        for pi in range(NPIECE):
                            if skip_ld:
                                if pi == 0:
                                    pc = ldp.tile([128, PIECE_CH * E], BF,
                                                  name="pc")
                                    nc.vector.memset(pc[:], 0.0)
                            else:
                                pc = ldp.tile([128, PIECE_CH * E], BF, name="pc")
                                nc.sync.dma_start(
                                    out=pc[:],
                                    in_=sdat[:, pi * PIECE_CH:(pi + 1) * PIECE_CH, :]
                                    .rearrange("p c e -> p (c e)"))
                            pc3 = pc[:].rearrange("p (c e) -> p c e", e=E)
                            for wl in range(4):           # windows in piece
                                w = pi * 4 + wl
                                pt = psA.tile([E, 128], DT, name="pt")
                                for k in range(WCAP):
                                    ci = w * WCAP + k
                                    cl = wl * WCAP + k
                                    oh = ohp.tile([128, 128], BF, name="oh")
                                    nc.vector.tensor_scalar(
                                        oh[:], iot[:], crel[br][:, ci:ci + 1],
                                        None, mybir.AluOpType.is_equal)
                                    nc.tensor.matmul(
                                        pt[:], pc3[:, cl, :], oh[:],
                                        start=(k == 0), stop=(k == WCAP - 1))
                                nc.scalar.activation(
                                    sumsT[br][:, w * 128:(w + 1) * 128], pt[:],
                                    mybir.ActivationFunctionType.Copy)
                    nc.sync.dma_start(
                        out=acc[(0 if br == "v" else E):
                                (E if br == "v" else 2 * E), :],
                        in_=sumsT[br][:])

            if skip_ar:
                nc.sync.dma_start(out=acc_red[:], in_=acc[:])
            else:
                nc.gpsimd.collective_compute(
                    "AllReduce", mybir.AluOpType.add,
                    ins=[acc[:]], outs=[acc_red[:]], replica_groups=rg)

            # ---------------- per-branch attention + table ----------------
            for br in ("v", "c"):
                bi = 0 if br == "v" else 1
                with tc.tile_pool(name=f"ap_{br}", bufs=1) as ap:
                    feaT = ap.tile([E, VN], DT, name="feaT")
                    nc.sync.dma_start(
                        out=feaT[:], in_=acc_red[bi * E:(bi + 1) * E, :])
                    invT = ap.tile([E, VN], DT, name="invT")
                    nc.sync.dma_start(out=invT[:], in_=ins[f"invT_{br}"][:])
                    nc.vector.tensor_tensor(
                        feaT[:], feaT[:], invT[:], mybir.AluOpType.mult)

                    # weights
                    wq = ap.tile([E, 128], DT, name="wq"); nc.sync.dma_start(out=wq[:], in_=ins[f"wqT_{br}"][:])
                    wk = ap.tile([E, 128], DT, name="wk"); nc.sync.dma_start(out=wk[:], in_=ins[f"wkT_{br}"][:])
                    wv = ap.tile([E, E], DT, name="wv"); nc.sync.dma_start(out=wv[:], in_=ins[f"wvT_{br}"][:])
                    wo = ap.tile([E, E], DT, name="wo"); nc.sync.dma_start(out=wo[:], in_=ins[f"woT_{br}"][:])
                    bq = ap.tile([128, 1], DT, name="bq"); nc.sync.dma_start(out=bq[:], in_=ins[f"bq_{br}"][:])
                    bk = ap.tile([128, 1], DT, name="bk"); nc.sync.dma_start(out=bk[:], in_=ins[f"bk_{br}"][:])
                    bv = ap.tile([E, 1], DT, name="bv"); nc.sync.dma_start(out=bv[:], in_=ins[f"bv_{br}"][:])
                    bo = ap.tile([E, 1], DT, name="bo"); nc.sync.dma_start(out=bo[:], in_=ins[f"bo_{br}"][:])
                    smq = ap.tile([E, QCH], DT, name="smq")
                    nc.sync.dma_start(out=smq[:], in_=ins[f"semq_{br}"][:])

                    ktile = ap.tile([128, VN], DT, name="ktile")
                    qtile = ap.tile([128, QCH], DT, name="qtile")
                    vtile = ap.tile([128, TC, 17 * H], DT, name="vtile")
                    with tc.tile_pool(name=f"pP_{br}", bufs=2,
                                      space=bass.MemorySpace.PSUM) as pP:
                        for ch in range(VN // 512):
                            kps = pP.tile([128, 512], DT, name="kps")
                            nc.tensor.matmul(
                                kps[:], wk[:], feaT[:, ch * 512:(ch + 1) * 512])
                            nc.vector.tensor_scalar_add(
                                ktile[:, ch * 512:(ch + 1) * 512], kps[:], bk[:])
                        qps = pP.tile([128, QCH], DT, name="kps")
                        nc.tensor.matmul(qps[:], wq[:], smq[:])
                        nc.vector.tensor_scalar_add(qtile[:], qps[:], bq[:])

                        for h in range(H):
                            nc.vector.memset(vtile[:, :, 17 * h:17 * h + 1], 1.0)
                        for a in range(TC):
                            vps = pP.tile([128, E], DT, name="vps")
                            nc.tensor.matmul(
                                vps[:], feaT[:, a * 128:(a + 1) * 128], wv[:])
                            nc.vector.tensor_copy(
                                vtile[:, a, :].rearrange(
                                    "p (h d) -> p h d", d=17)[:, :, 1:17],
                                vps[:].rearrange("p (h d) -> p h d", d=16))

                    attnT = ap.tile([E, QCH], DT, name="attnT")
                    with tc.tile_pool(name=f"pA_{br}", bufs=1,
                                      space=bass.MemorySpace.PSUM) as pA:
                        avps = [pA.tile([17, QCH], DT, name=f"avps{h}")
                                for h in range(H)]
                        with tc.tile_pool(name=f"pS_{br}", bufs=1,
                                          space=bass.MemorySpace.PSUM) as pS, \
                             tc.tile_pool(name=f"eS_{br}", bufs=2) as eS:
                            for a in ([0] if skip_at else range(TC)):
                                scf = pS.tile([128, H * QCH], DT, name="scf")
                                for h in range(H):
                                    nc.tensor.matmul(
                                        scf[:, h * QCH:(h + 1) * QCH],
                                        ktile[32 * h:32 * h + 32,
                                              a * 128:(a + 1) * 128],
                                        qtile[32 * h:32 * h + 32, :],
                                        tile_position=(32 * h, 0))
                                exf = eS.tile([128, H * QCH], DT, name="exf")
                                nc.scalar.activation(
                                    exf[:], scf[:],
                                    mybir.ActivationFunctionType.Exp)
                                for h in range(H):
                                    nc.tensor.matmul(
                                        avps[h][:],
                                        vtile[:, a, 17 * h:17 * h + 17],
                                        exf[:, h * QCH:(h + 1) * QCH],
                                        start=(a == 0),
                                        stop=(a == TC - 1 or skip_at),
                                        skip_group_check=True)

                        with tc.tile_pool(name=f"pN_{br}", bufs=1,
                                          space=bass.MemorySpace.PSUM) as pN, \
                             tc.tile_pool(name=f"eN_{br}", bufs=1) as eN:
                            one17 = eN.tile([1, 17], DT, name="one17")
                            nc.vector.memset(one17[:], 1.0)
                            for h in range(H):
                                rec = eN.tile([1, QCH], DT, name="rec", bufs=2)
                                nc.vector.reciprocal(rec[:], avps[h][0:1, :])
                                rbc = pN.tile([17, QCH], DT, name="rbc", bufs=2)
                                nc.tensor.matmul(rbc[:], one17[:], rec[:])
                                rbs = eN.tile([17, QCH], DT, name="rbs", bufs=2)
                                nc.vector.tensor_copy(rbs[:], rbc[:])
                                at_ = eN.tile([17, QCH], DT, name="at", bufs=2)
                                nc.vector.tensor_tensor(
                                    at_[:], avps[h][0:17, :], rbs[:],
                                    mybir.AluOpType.mult)
                                nc.sync.dma_start(
                                    out=attnT[16 * h:16 * h + 16, :],
                                    in_=at_[1:17, :])

                    # + bv, out-proj, transpose to rows, quad-replicate bf16
                    nc.vector.tensor_scalar_add(attnT[:], attnT[:], bv[:])
                    dupt = ap.tile([128, QCH // 128, QELEM], BF, name="dupt")
                    with tc.tile_pool(name=f"pF_{br}", bufs=2,
                                      space=bass.MemorySpace.PSUM) as pF:
                        fps = pF.tile([E, QCH], DT, name="fps", bufs=1)
                        nc.tensor.matmul(fps[:], wo[:], attnT[:])
                        fT = ap.tile([E, QCH], DT, name="fT")
                        nc.vector.tensor_scalar_add(fT[:], fps[:], bo[:])
                        for i in range(QCH // 128):
                            tp = pF.tile([128, E], DT, name="tp")
                            nc.tensor.transpose(
                                tp[:], fT[:, i * 128:(i + 1) * 128],
                                idt[0:E, 0:E])
                            for d in range(4):
                                nc.vector.tensor_copy(
                                    dupt[:, i, d * E:(d + 1) * E], tp[:])
                    nc.sync.dma_start(
                        out=tbl_own[bi * QCH:(bi + 1) * QCH, :].rearrange(
                            "(i p) d -> p i d", p=128),
                        in_=dupt[:])

            nc.gpsimd.collective_compute(
                "AllGather", mybir.AluOpType.bypass,
                ins=[tbl_own[:]], outs=[tbl_all[:]], replica_groups=rg)

            # ---------------- output quad-gather ----------------
            qn = [0]
            with tc.tile_pool(name="gp", bufs=3) as gp, \
                 tc.tile_pool(name="gip", bufs=1) as gip:
                git = {}
                for br in ("v", "c"):
                    git[br] = gip.tile([128, nq_slots // 16], I16,
                                       name=f"git_{br}")
                    nc.sync.dma_start(out=git[br][:], in_=ins[f"gidx_{br}"][:])
                for br_i, br in enumerate(("v", "c")):
                    for pi in range(n_gpiece):
                        gt = gp.tile(
                            [128, CALLS_PER_PIECE * (GC // 128) * QELEM], BF,
                            name="gt")
                        g3 = gt[:].rearrange("p (a f) -> p a f", f=QELEM)
                        if skip_ga:
                            nc.vector.memset(gt[:], 0.0)
                        else:
                            for ci in range(CALLS_PER_PIECE):
                                q0 = pi * CALLS_PER_PIECE * GC + ci * GC
                                nc.gpsimd.dma_gather(
                                    g3[:, ci * (GC // 128):
                                       (ci + 1) * (GC // 128), :],
                                    tbl_all[:],
                                    git[br][:, q0 // 16:(q0 + GC) // 16],
                                    GC, GC, QELEM,
                                    queue_num=qn[0] % 4)
                                qn[0] += 1
                        nc.sync.dma_start(out=out_ext[br_i, pi], in_=g3)
    nc.compile()
    return nc


@functools.cache
def _compiled(nq_slots: int):
    return _build(nq_slots)


# ------------------------- host-side preparation -------------------------

def _wrap_idx(idx: np.ndarray) -> np.ndarray:
    n = idx.shape[0]
    w = np.ascontiguousarray(idx.reshape(n // 16, 16).T).astype(np.int16)
    return np.tile(w, (8, 1))


def _branch_weights(in_w, in_b, out_w, out_b):
    in_w = np.asarray(in_w, np.float32)
    in_b = np.asarray(in_b, np.float32)
    wq, wk, wv = in_w[:E], in_w[E:2 * E], in_w[2 * E:]
    bq, bk, bv = in_b[:E], in_b[E:2 * E], in_b[2 * E:]
    scale = np.float32(1.0 / np.sqrt(HD))
    wqT_pad = np.zeros((E, 128), np.float32)
    wkT_pad = np.zeros((E, 128), np.float32)
    bq_pad = np.zeros((128, 1), np.float32)
    bk_pad = np.zeros((128, 1), np.float32)
    for h in range(H):
        for j in range(HD):
            wqT_pad[:, 32 * h + j] = wq[HD * h + j] * scale
            wkT_pad[:, 32 * h + j] = wk[HD * h + j]
            bq_pad[32 * h + j, 0] = bq[HD * h + j] * scale
            bk_pad[32 * h + j, 0] = bk[HD * h + j]
    return {
        "wqT": wqT_pad, "wkT": wkT_pad,
        "wvT": np.ascontiguousarray(wv.T),
        "woT": np.ascontiguousarray(np.asarray(out_w, np.float32).T),
        "bq": bq_pad, "bk": bk_pad,
        "bv": bv.reshape(E, 1).astype(np.float32),
        "bo": np.asarray(out_b, np.float32).reshape(E, 1),
    }


def _prep_phase_a(s_bf: np.ndarray, cls: np.ndarray):
    """Sort shard rows by class, deal into 32 windows x 33 chunks x 128 rows.
    Returns (sdat [128,NCHK,E] bf16, crel [128,NCHK] f32)."""
    n = cls.shape[0]
    order = np.argsort(cls, kind="stable")
    scls = cls[order].astype(np.int64)
    win = scls >> 7
    wstart = np.searchsorted(win, np.arange(WIN))
    wcount = np.diff(np.r_[wstart, n])
    if wcount.max() > WCAP * 128:
        raise RuntimeError(f"window overflow: {wcount.max()} > {WCAP * 128}")
    rank = np.arange(n) - wstart[win]
    slot = win * (WCAP * 128) + rank
    rows = np.zeros((NSLOT, E), BF_NP)
    rows[slot] = s_bf[order]
    crel_f = np.zeros(NSLOT, np.float32)
    crel_f[slot] = (scls & 127).astype(np.float32)
    sdat = np.ascontiguousarray(rows.reshape(NCHK, 128, E).transpose(1, 0, 2))
    crel = np.ascontiguousarray(crel_f.reshape(NCHK, 128).T)
    return sdat, crel


def _prep_quads(cls: np.ndarray, br_i: int, nq_slots: int):
    """Group same-class rows (sorted order) into quads.
    Returns (gidx [128, nq_slots/16] int16, q_global [n], qslot [n])."""
    n = cls.shape[0]
    order = np.argsort(cls, kind="stable")
    scls = cls[order].astype(np.int64)
    cnt = np.bincount(cls, minlength=VN)
    cstart = np.r_[0, np.cumsum(cnt)]
    rank = np.arange(n) - cstart[scls]
    nquad = (cnt + 3) // 4
    qbase = np.r_[0, np.cumsum(nquad)]
    q_of_sorted = qbase[scls] + (rank >> 2)
    nq_real = int(qbase[-1])
    assert nq_real <= nq_slots, (nq_real, nq_slots)
    qcls = np.zeros(nq_slots, np.int64)
    qcls[q_of_sorted] = scls
    tblrow = (qcls >> 9) * (2 * QCH) + br_i * QCH + (qcls & (QCH - 1))
    # inverse map: original row order[i] -> (quad q_of_sorted[i], slot rank%4)
    q_global = np.empty(n, np.int64)
    q_global[order] = q_of_sorted
    qslot = np.empty(n, np.int64)
    qslot[order] = rank & 3
    return _wrap_idx(tblrow), q_global, qslot


def _make_plan(v_class, c_class):
    """nq_slots: max quads over cores/branches, rounded to a piece."""
    nq_max = 0
    for cls_all in (v_class, c_class):
        for core in range(N_CORES):
            cls = cls_all[core * SHARD:(core + 1) * SHARD]
            cnt = np.bincount(cls, minlength=VN)
            nq_max = max(nq_max, int(((cnt + 3) // 4).sum()))
    piece = CALLS_PER_PIECE * GC
    return ((nq_max + piece - 1) // piece) * piece


def _make_in_maps(v_s, c_s, v_sem, c_sem, v_class, c_class,
                  v_in_w, v_in_b, v_out_w, v_out_b,
                  c_in_w, c_in_b, c_out_w, c_out_b, nq_slots):
    v_class = np.asarray(v_class, np.int32)
    c_class = np.asarray(c_class, np.int32)
    v_bf = np.asarray(v_s, np.float32).astype(BF_NP)
    c_bf = np.asarray(c_s, np.float32).astype(BF_NP)
    v_semT = np.ascontiguousarray(np.asarray(v_sem, np.float32).T)
    c_semT = np.ascontiguousarray(np.asarray(c_sem, np.float32).T)
    wts = {"v": _branch_weights(v_in_w, v_in_b, v_out_w, v_out_b),
           "c": _branch_weights(c_in_w, c_in_b, c_out_w, c_out_b)}
    ident = np.eye(128, dtype=np.float32)
    iotaF = np.ascontiguousarray(
        np.broadcast_to(np.arange(128, dtype=np.float32), (128, 128))
    ).astype(BF_NP)
    invT = {}
    for br, cls in (("v", v_class), ("c", c_class)):
        cnt = np.bincount(cls, minlength=VN).astype(np.float32)
        inv = (1.0 / (cnt + 1e-8)).astype(np.float32)
        invT[br] = np.ascontiguousarray(
            np.broadcast_to(inv[None, :], (E, VN)))
    in_maps = []
    unmaps = []
    for core in range(N_CORES):
        b0 = core * SHARD
        m = {"ident": ident, "iotaF": iotaF}
        um = {}
        for br_i, (br, s_bf, cls_all, semT) in enumerate((
                ("v", v_bf, v_class, v_semT), ("c", c_bf, c_class, c_semT))):
            cls = cls_all[b0:b0 + SHARD]
            sdat, crel = _prep_phase_a(s_bf[b0:b0 + SHARD], cls)
            gidx, q_global, qslot = _prep_quads(cls, br_i, nq_slots)
            m[f"sdat_{br}"] = sdat
            m[f"crel_{br}"] = crel
            m[f"gidx_{br}"] = gidx
            m[f"invT_{br}"] = invT[br]
            m[f"semq_{br}"] = np.ascontiguousarray(
                semT[:, core * QCH:(core + 1) * QCH])
            for k, vv in wts[br].items():
                m[f"{k}_{br}"] = vv
            um[br] = (q_global, qslot)
        in_maps.append(m)
        unmaps.append(um)
    return in_maps, unmaps


def _unpack_out(res_results, unmaps, nq_slots):
    """out_ext [2, n_gpiece, 128, CP*8, QELEM] bf16 -> full f32 outputs."""
    n_gpiece = nq_slots // (CALLS_PER_PIECE * GC)
    v_out = np.empty((NV, E), np.float32)
    c_out = np.empty((NV, E), np.float32)
    for core in range(N_CORES):
        o = np.asarray(res_results[core]["out"])
        if o.dtype != BF_NP:
            o = o.view(BF_NP)
        o = o.reshape(2, n_gpiece, 128, CALLS_PER_PIECE, GC // 128, QELEM)
        # quad q = piece*CP*GC + ci*GC + jj*128 + p  at o[br, piece, p, ci, jj]
        quads = np.ascontiguousarray(
            o.transpose(0, 1, 3, 4, 2, 5)).reshape(
                2, nq_slots, 4, E).astype(np.float32)
        for br_i, (br, out) in enumerate((("v", v_out), ("c", c_out))):
            q_global, qslot = unmaps[core][br]
            out[core * SHARD:(core + 1) * SHARD] = \
                quads[br_i, q_global, qslot, :]
    return v_out, c_out


def kernel(v_s, c_s, v_sem, c_sem, v_class, c_class,
           v_in_w, v_in_b, v_out_w, v_out_b,
           c_in_w, c_in_b, c_out_w, c_out_b):
    nq_slots = _make_plan(np.asarray(v_class, np.int32),
                          np.asarray(c_class, np.int32))
    in_maps, unmaps = _make_in_maps(
        v_s, c_s, v_sem, c_sem, v_class, c_class,
        v_in_w, v_in_b, v_out_w, v_out_b,
        c_in_w, c_in_b, c_out_w, c_out_b, nq_slots)
    nc = _compiled(nq_slots)
    res = run_bass_kernel_spmd(nc, in_maps, core_ids=list(range(N_CORES)))
    return _unpack_out(res.results, unmaps, nq_slots)


# exposed for test.py timing
def prepare_in_maps(inputs):
    sig = ["v_s", "c_s", "v_sem", "c_sem", "v_class", "c_class",
           "v_in_w", "v_in_b", "v_out_w", "v_out_b",
           "c_in_w", "c_in_b", "c_out_w", "c_out_b"]
    kw = {k: inputs[k] for k in sig}
    nq_slots = _make_plan(np.asarray(kw["v_class"], np.int32),
                          np.asarray(kw["c_class"], np.int32))
    in_maps, _ = _make_in_maps(**kw, nq_slots=nq_slots)
    return _compiled(nq_slots), in_maps
